# revision 12
# baseline (speedup 1.0000x reference)
"""DCGRU cell Trainium2 kernel (Bass/Tile), data-parallel over batch on 8 cores.

Math (per core, local batch BL=4):
  x0 = concat([inputs, states], -1)                    [node, F=BL*66] layout
  x1 = A @ x0          (pass 1)
  x2 = 2 A @ x1 - x0   (pass 2, fused)
  z_ru = sum_m x_m W_m + b    (feature-contraction via on-chip transposes)
  r, u = sigmoid(z_ru) split
  y0 = concat([inputs, r*states])   (reuses x0 buffer, only state cols rewritten)
  y1 = A @ y0          (pass 3)
  y2 = 2 A @ y1 - y0   (pass 4)
  c = tanh(sum_m y_m Wc_m + bc)
  out = c + u * (states - c)

A is fp8 (pre-scaled by 4096 to avoid subnormal flush) and arrives SHARDED:
each core receives 1/8 of the m-tiles and an on-device AllGather assembles
the full transposed+tiled A in local HBM before the diffusion passes stream
it. x tensors stay resident in SBUF in bf16 with fp8 shadows cast on-chip.
All A-matmuls are fp8 DoubleRow with fp32 PSUM accumulation. u and r*states
round-trip through internal DRAM; outputs are written bf16.
"""
import sys

sys.path.insert(0, "/opt/trn_rl_repo")

import numpy as np
import ml_dtypes

BF16 = ml_dtypes.bfloat16
F8 = ml_dtypes.float8_e4m3fn

# problem constants
N_REAL = 10000
B_REAL = 32
D_IN = 2
H = 64
G = D_IN + H            # 66
RU = 2 * H              # 128
N_CORES = 8
BL = B_REAL // N_CORES  # 4 local batch
NT = (N_REAL + 127) // 128        # 79 k tiles
MTS = (NT + N_CORES - 1) // N_CORES * N_CORES  # 80 m tiles (padded to shard)
MS = MTS // N_CORES     # 10 m tiles per core shard
GRP = 4                 # node tiles per linear-stage group

A_SCALE = 4096.0  # fp8 pre-scale for A (avoids subnormal flush)


def build_nc(repeats=1, gather_in_loop=True):
    import concourse.bacc as bacc
    import concourse.mybir as mybir
    import concourse.tile as tile

    f32 = mybir.dt.float32
    bf = mybir.dt.bfloat16
    f8 = mybir.dt.float8e4
    ALU = mybir.AluOpType
    ACT = mybir.ActivationFunctionType

    NTP = NT * 128
    F = BL * G                # 264
    FP = (F + 15) // 16 * 16  # 272 fp8 tile stride (DoubleRow step%16==0)
    inv_s = 1.0 / A_SCALE

    nc = bacc.Bacc(
        "TRN2", target_bir_lowering=False, debug=False, num_devices=N_CORES
    )

    a_t = nc.dram_tensor("a_t", [MS, 128, NT, 128], f8, kind="ExternalInput")
    x0_d = nc.dram_tensor("x0", [NTP, F], bf, kind="ExternalInput")
    x0t_d = nc.dram_tensor("x0t", [BL, G, NTP], bf, kind="ExternalInput")
    wru_d = nc.dram_tensor("wru", [3, G, RU], bf, kind="ExternalInput")
    wc12_d = nc.dram_tensor("wc12", [2, G, H], bf, kind="ExternalInput")
    wcin_d = nc.dram_tensor("wcin", [D_IN, H], bf, kind="ExternalInput")
    wcst_d = nc.dram_tensor("wcst", [H, H], bf, kind="ExternalInput")
    bru_d = nc.dram_tensor("bru", [RU, 1], f32, kind="ExternalInput")
    bc_d = nc.dram_tensor("bc", [H, 1], f32, kind="ExternalInput")
    id_d = nc.dram_tensor("ident", [128, 128], bf, kind="ExternalInput")
    out_d = nc.dram_tensor("outt", [BL, H, NTP], bf, kind="ExternalOutput")
    u_d = nc.dram_tensor("u_i", [BL, H, NTP], bf)
    rs_d = nc.dram_tensor("rs_i", [BL, H, NTP], bf)
    a_bnc = nc.dram_tensor("a_bnc", [MS, 128, NT, 128], f8)
    # one gathered stripe per shard-local m index: a_gs[i][c] = m-tile c*MS+i
    a_gs = [
        nc.dram_tensor(f"a_g{i}", [N_CORES, 128, NT, 128], f8,
                       addr_space="Shared")
        for i in range(MS)
    ]

    groups = [(g, min(GRP, NT - g)) for g in range(0, NT, GRP)]
    # pass-1 visits m in stripe order so compute starts after stripe 0 lands
    stripe_order = [c * MS + i for i in range(MS) for c in range(N_CORES)
                    if c * MS + i < NT]

    with tile.TileContext(nc) as tc:
        with (
            tc.tile_pool(name="persist", bufs=1) as persist,
            tc.tile_pool(name="apool", bufs=3) as apool,
            tc.tile_pool(name="xtpool", bufs=2) as xtpool,
            tc.tile_pool(name="spool", bufs=3) as spool,
            tc.tile_pool(name="pmm", bufs=4, space="PSUM") as pmm,
            tc.tile_pool(name="ptr", bufs=2, space="PSUM") as ptr,
            tc.tile_pool(name="pz", bufs=2, space="PSUM") as pz,
        ):
            x0_buf = persist.tile([128, NT * F], bf, tag="x0b")
            x1_buf = persist.tile([128, NT * F], bf, tag="x1b")
            x0f8_buf = persist.tile([128, NT * FP], f8, tag="x0f8b")
            x1f8_buf = persist.tile([128, NT * FP], f8, tag="x1f8b")
            wru_sb = persist.tile([G, 3 * RU], bf, tag="wru")
            wc12_sb = persist.tile([G, 2 * H], bf, tag="wc12")
            wcin_sb = persist.tile([G, H], bf, tag="wcin")  # rows H:G used
            wcst_sb = persist.tile([H, H], bf, tag="wcst")
            ident = persist.tile([128, 128], bf, tag="ident")
            bru_sb = persist.tile([RU, 1], f32, tag="bru")
            bc_sb = persist.tile([H, 1], f32, tag="bc")

            def load_x0():
                for k in range(NT):
                    nc.sync.dma_start(
                        x0_buf[:, k * F:(k + 1) * F],
                        x0_d[k * 128:(k + 1) * 128, :],
                    )
                    nc.vector.tensor_copy(
                        x0f8_buf[:, k * FP:k * FP + F],
                        x0_buf[:, k * F:(k + 1) * F],
                    )

            def gather_a():
                nc.sync.dma_start(a_bnc[:], a_t[:])
                for i in range(MS):
                    nc.gpsimd.collective_compute(
                        "AllGather", mybir.AluOpType.bypass,
                        replica_groups=[list(range(N_CORES))],
                        ins=[a_bnc[i:i + 1].opt()], outs=[a_gs[i][:].opt()],
                    )

            load_x0()
            for m in range(3):
                nc.sync.dma_start(wru_sb[:, m * RU:(m + 1) * RU], wru_d[m])
            for m in range(2):
                nc.sync.dma_start(wc12_sb[:, m * H:(m + 1) * H], wc12_d[m])
            nc.sync.dma_start(wcin_sb[H:G, :], wcin_d[:])
            nc.sync.dma_start(wcst_sb[:], wcst_d[:])
            nc.sync.dma_start(ident[:], id_d[:])
            nc.sync.dma_start(bru_sb[:], bru_d[:])
            nc.sync.dma_start(bc_sb[:], bc_d[:])

            def diffusion_pass(rhs_buf, handler, m_order=None):
                # rhs_buf is the fp8 (FP-strided) shadow buffer
                for m in (m_order if m_order is not None else range(NT)):
                    ps = pmm.tile([128, F], f32, tag="mmps")
                    # whole-m A row: one 1.29MB DMA (78%+ DMA efficiency)
                    at = apool.tile([128, NT * 128], f8, tag="astage")
                    nc.sync.dma_start(at[:], a_gs[m % MS][m // MS])
                    for kk in range(0, NT - 1, 2):
                        lhsT = at[
                            :, kk * 128:(kk + 2) * 128
                        ].rearrange("p (two m2) -> p two m2", two=2)
                        rhs = rhs_buf[
                            :, kk * FP:(kk + 2) * FP
                        ].rearrange("p (two f) -> p two f", two=2)[
                            :, :, 0:F
                        ]
                        nc.tensor.matmul(
                            ps[:], lhsT, rhs,
                            start=(kk == 0), stop=(kk + 2 == NT),
                            perf_mode=mybir.MatmulPerfMode.DoubleRow,
                        )
                    if NT % 2:
                        kk = NT - 1
                        nc.tensor.matmul(
                            ps[:],
                            at[:, kk * 128:(kk + 1) * 128],
                            rhs_buf[:, kk * FP:kk * FP + F],
                            start=(kk == 0),
                            stop=True,
                        )
                    handler(m, ps)

            def h_copy(m, ps):
                nc.vector.tensor_scalar_mul(
                    x1_buf[:, m * F:(m + 1) * F], ps[:], inv_s
                )
                nc.vector.tensor_copy(
                    x1f8_buf[:, m * FP:m * FP + F],
                    x1_buf[:, m * F:(m + 1) * F],
                )

            stage = {}

            def transpose_lo_hi(m, j, lo_buf_slice, hi_tile):
                # lo = previous diffusion output [128, F]; hi = 2*A@lo - base
                for b in range(BL):
                    tp = ptr.tile([128, 128], bf, tag="trp")
                    nc.tensor.transpose(
                        tp[:G, :], lo_buf_slice[:, b * G:(b + 1) * G], ident[:]
                    )
                    nc.vector.tensor_copy(
                        stage[(1, b)][:, j * 128:(j + 1) * 128], tp[:G, :]
                    )
                    tp2 = ptr.tile([128, 128], bf, tag="trp")
                    nc.tensor.transpose(
                        tp2[:G, :], hi_tile[:, b * G:(b + 1) * G], ident[:]
                    )
                    nc.vector.tensor_copy(
                        stage[(2, b)][:, j * 128:(j + 1) * 128], tp2[:G, :]
                    )

            def h_gconv1(m, ps):
                gi, j = m // GRP, m % GRP
                g0, gn = groups[gi]
                if j == 0:
                    for b in range(BL):
                        stage[(1, b)] = xtpool.tile(
                            [G, GRP * 128], bf, tag=f"s1_{b}", name=f"s1_{b}"
                        )
                        stage[(2, b)] = xtpool.tile(
                            [G, GRP * 128], bf, tag=f"s2_{b}", name=f"s2_{b}"
                        )
                x2t = spool.tile([128, F], bf, tag="x2tmp")
                nc.vector.scalar_tensor_tensor(
                    x2t[:], ps[:], 2.0 * inv_s, x0_buf[:, m * F:(m + 1) * F],
                    op0=ALU.mult, op1=ALU.subtract,
                )
                transpose_lo_hi(m, j, x1_buf[:, m * F:(m + 1) * F], x2t)
                if j == gn - 1:
                    W = gn * 128
                    c0 = g0 * 128
                    for b in range(BL):
                        x0t_t = spool.tile([G, GRP * 128], bf, tag="x0ts")
                        nc.sync.dma_start(x0t_t[:, :W], x0t_d[b, :, c0:c0 + W])
                        zp = pz.tile([128, 512], f32, tag="zps")
                        nc.tensor.matmul(
                            zp[:, :W], wru_sb[:, 0:RU], x0t_t[:, :W],
                            start=True, stop=False,
                        )
                        nc.tensor.matmul(
                            zp[:, :W], wru_sb[:, RU:2 * RU],
                            stage[(1, b)][:, :W], start=False, stop=False,
                        )
                        nc.tensor.matmul(
                            zp[:, :W], wru_sb[:, 2 * RU:3 * RU],
                            stage[(2, b)][:, :W], start=False, stop=True,
                        )
                        rt = spool.tile([H, GRP * 128], bf, tag="rt")
                        nc.scalar.activation(
                            rt[:, :W], zp[0:H, :W], ACT.Sigmoid,
                            bias=bru_sb[0:H],
                        )
                        ut = spool.tile([H, GRP * 128], bf, tag="ut")
                        nc.scalar.activation(
                            ut[:, :W], zp[H:RU, :W], ACT.Sigmoid,
                            bias=bru_sb[H:RU],
                        )
                        nc.sync.dma_start(u_d[b, :, c0:c0 + W], ut[:, :W])
                        rs = spool.tile([H, GRP * 128], bf, tag="rs")
                        nc.vector.tensor_mul(
                            rs[:, :W], rt[:, :W], x0t_t[0:H, :W]
                        )
                        nc.sync.dma_start(rs_d[b, :, c0:c0 + W], rs[:, :W])
                        for jj in range(gn):
                            m2 = g0 + jj
                            tpb = ptr.tile([128, 128], bf, tag="trp")
                            nc.tensor.transpose(
                                tpb[:, :H], rs[:, jj * 128:(jj + 1) * 128],
                                ident[:H, :H],
                            )
                            nc.vector.tensor_copy(
                                x0_buf[:, m2 * F + b * G:
                                       m2 * F + b * G + H],
                                tpb[:, :H],
                            )
                            nc.vector.tensor_copy(
                                x0f8_buf[:, m2 * FP + b * G:
                                         m2 * FP + b * G + H],
                                tpb[:, :H],
                            )

            def h_gconv2(m, ps):
                gi, j = m // GRP, m % GRP
                g0, gn = groups[gi]
                if j == 0:
                    for b in range(BL):
                        stage[(1, b)] = xtpool.tile(
                            [G, GRP * 128], bf, tag=f"s1_{b}", name=f"s1_{b}"
                        )
                        stage[(2, b)] = xtpool.tile(
                            [G, GRP * 128], bf, tag=f"s2_{b}", name=f"s2_{b}"
                        )
                y2t = spool.tile([128, F], bf, tag="x2tmp")
                nc.vector.scalar_tensor_tensor(
                    y2t[:], ps[:], 2.0 * inv_s, x0_buf[:, m * F:(m + 1) * F],
                    op0=ALU.mult, op1=ALU.subtract,
                )
                transpose_lo_hi(m, j, x1_buf[:, m * F:(m + 1) * F], y2t)
                if j == gn - 1:
                    W = gn * 128
                    c0 = g0 * 128
                    for b in range(BL):
                        x0t_t = spool.tile([G, GRP * 128], bf, tag="x0ts")
                        nc.sync.dma_start(x0t_t[:, :W], x0t_d[b, :, c0:c0 + W])
                        rs_t = spool.tile([H, GRP * 128], bf, tag="rsin")
                        nc.sync.dma_start(rs_t[:, :W], rs_d[b, :, c0:c0 + W])
                        zc = pz.tile([128, 512], f32, tag="zps")
                        nc.tensor.matmul(
                            zc[:H, :W], wcin_sb[H:G, :], x0t_t[H:G, :W],
                            start=True, stop=False,
                        )
                        nc.tensor.matmul(
                            zc[:H, :W], wcst_sb[:], rs_t[:, :W],
                            start=False, stop=False,
                        )
                        nc.tensor.matmul(
                            zc[:H, :W], wc12_sb[:, 0:H],
                            stage[(1, b)][:, :W], start=False, stop=False,
                        )
                        nc.tensor.matmul(
                            zc[:H, :W], wc12_sb[:, H:2 * H],
                            stage[(2, b)][:, :W], start=False, stop=True,
                        )
                        ct = spool.tile([H, GRP * 128], f32, tag="ct")
                        nc.scalar.activation(
                            ct[:, :W], zc[:H, :W], ACT.Tanh, bias=bc_sb[:]
                        )
                        ut2 = spool.tile([H, GRP * 128], bf, tag="uin")
                        nc.sync.dma_start(ut2[:, :W], u_d[b, :, c0:c0 + W])
                        t1 = spool.tile([H, GRP * 128], f32, tag="t1")
                        nc.vector.tensor_sub(
                            t1[:, :W], x0t_t[0:H, :W], ct[:, :W]
                        )
                        nc.vector.tensor_mul(t1[:, :W], t1[:, :W], ut2[:, :W])
                        op = spool.tile([H, GRP * 128], bf, tag="outp")
                        nc.vector.tensor_add(op[:, :W], t1[:, :W], ct[:, :W])
                        nc.sync.dma_start(out_d[b, :, c0:c0 + W], op[:, :W])

            if not gather_in_loop:
                gather_a()
            for rep in range(repeats):
                if gather_in_loop:
                    gather_a()
                if rep > 0:
                    load_x0()                      # x0_buf was turned into y0
                diffusion_pass(x0f8_buf, h_copy, m_order=stripe_order)
                                                   # pass 1: x1 = A x0
                diffusion_pass(x1f8_buf, h_gconv1) # pass 2: x2 + gconv1 linear
                diffusion_pass(x0f8_buf, h_copy)   # pass 3: y1 = A y0
                diffusion_pass(x1f8_buf, h_gconv2) # pass 4: y2 + gconv2 + out

    nc.compile()
    return nc


def _f8_lut():
    """LUT mapping bf16 bit patterns -> fp8 bits of (value * A_SCALE)."""
    v = np.arange(65536, dtype=np.uint16).view(BF16).astype(np.float32)
    with np.errstate(invalid="ignore", over="ignore"):
        return (v * A_SCALE).astype(F8).view(np.uint8)


def prep_a(supports):
    """Full A -> fp8, transposed + tiled: a[m,p,k,q] = A[m*128+q, k*128+p]."""
    N = supports.shape[0]
    lut = _f8_lut()
    a_bf = supports.astype(BF16)                      # fast truncation cast
    a8 = lut[a_bf.view(np.uint16)]                    # fp8 bits, row-major A
    ap = np.zeros((MTS * 128, NT * 128), np.uint8)    # pad rows(m) and cols(k)
    ap[:N, :N] = a8
    out = np.empty((MTS, 128, NT, 128), np.uint8)
    for m in range(MTS):                              # blocked: ~8x faster
        out[m] = ap[m * 128:(m + 1) * 128].reshape(128, NT, 128).transpose(2, 1, 0)
    return out.view(F8)


def prep_shared(W_ru, b_ru, W_c, b_c):
    # feature order is [states(64); inputs(2)] throughout the kernel
    perm = list(range(D_IN, G)) + list(range(D_IN))
    wru_r = np.ascontiguousarray(
        W_ru.reshape(G, 3, RU)[perm].transpose(1, 0, 2)
    ).astype(BF16)
    wc_r = np.ascontiguousarray(
        W_c.reshape(G, 3, H)[perm].transpose(1, 0, 2)
    ).astype(BF16)
    return {
        "wru": wru_r,
        "wc12": np.ascontiguousarray(wc_r[1:3]),
        "wcin": np.ascontiguousarray(wc_r[0, H:G]),
        "wcst": np.ascontiguousarray(wc_r[0, 0:H]),
        "bru": b_ru.reshape(RU, 1).astype(np.float32),
        "bc": b_c.reshape(H, 1).astype(np.float32),
        "ident": np.eye(128, dtype=BF16),
    }


def prep_core(x_cat):
    # x_cat: [BL, N, G] f32 for this core's batch slice
    NTP = NT * 128
    N = x_cat.shape[1]
    F = BL * G
    x0 = np.zeros((NTP, F), BF16)
    x0[:N] = x_cat.transpose(1, 0, 2).reshape(N, F)
    x0t = np.zeros((BL, G, NTP), BF16)
    x0t[:, :, :N] = x_cat.transpose(0, 2, 1)
    return {"x0": x0, "x0t": x0t}


_NC_CACHE = {}


def _get_nc(repeats=1):
    if repeats not in _NC_CACHE:
        _NC_CACHE[repeats] = build_nc(repeats=repeats)
    return _NC_CACHE[repeats]


def make_in_maps(inputs, states, supports, W_ru, b_ru, W_c, b_c):
    inputs = np.asarray(inputs, np.float32)
    states = np.asarray(states, np.float32)
    supports = np.asarray(supports, np.float32)
    a_full = prep_a(supports)
    shared = prep_shared(
        np.asarray(W_ru, np.float32), np.asarray(b_ru, np.float32),
        np.asarray(W_c, np.float32), np.asarray(b_c, np.float32),
    )
    x_cat = np.concatenate([states, inputs], -1)  # [B, N, G], states-first
    in_maps = []
    for c in range(N_CORES):
        m = dict(shared)
        m["a_t"] = a_full[c * MS:(c + 1) * MS]
        m.update(prep_core(x_cat[c * BL:(c + 1) * BL]))
        in_maps.append(m)
    return in_maps


def _run(inputs, states, supports, W_ru, b_ru, W_c, b_c):
    from concourse.bass_utils import run_bass_kernel_spmd

    B, N, _ = np.asarray(inputs).shape
    nc = _get_nc(repeats=1)
    in_maps = make_in_maps(inputs, states, supports, W_ru, b_ru, W_c, b_c)
    res = run_bass_kernel_spmd(nc, in_maps, list(range(N_CORES)))
    out = np.empty((B, N, H), np.float32)
    for c in range(N_CORES):
        o = res.results[c]["outt"]          # [BL, H, NTP] bf16
        out[c * BL:(c + 1) * BL] = o.transpose(0, 2, 1)[:, :N, :].astype(np.float32)
    return out, res


def kernel(**kw):
    out, _ = _run(
        kw["inputs"], kw["states"], kw["supports"],
        kw["W_ru"], kw["b_ru"], kw["W_c"], kw["b_c"],
    )
    return out


# revision 15
# speedup vs baseline: 1.1806x; 1.1806x over previous
"""DCGRU cell Trainium2 kernel (Bass/Tile), data-parallel over batch on 8 cores.

Math (per core, local batch BL=4):
  x0 = concat([inputs, states], -1)                    [node, F=BL*66] layout
  x1 = A @ x0          (pass 1)
  x2 = 2 A @ x1 - x0   (pass 2, fused)
  z_ru = sum_m x_m W_m + b    (feature-contraction via on-chip transposes)
  r, u = sigmoid(z_ru) split
  y0 = concat([inputs, r*states])   (reuses x0 buffer, only state cols rewritten)
  y1 = A @ y0          (pass 3)
  y2 = 2 A @ y1 - y0   (pass 4)
  c = tanh(sum_m y_m Wc_m + bc)
  out = c + u * (states - c)

A is fp8 (pre-scaled by 4096 to avoid subnormal flush) and arrives SHARDED:
each core receives 1/8 of the m-tiles and an on-device AllGather assembles
the full transposed+tiled A in local HBM before the diffusion passes stream
it. x tensors stay resident in SBUF in bf16 with fp8 shadows cast on-chip.
All A-matmuls are fp8 DoubleRow with fp32 PSUM accumulation. u and r*states
round-trip through internal DRAM; outputs are written bf16.
"""
import sys

sys.path.insert(0, "/opt/trn_rl_repo")

import numpy as np
import ml_dtypes

BF16 = ml_dtypes.bfloat16
F8 = ml_dtypes.float8_e4m3fn

# problem constants
N_REAL = 10000
B_REAL = 32
D_IN = 2
H = 64
G = D_IN + H            # 66
RU = 2 * H              # 128
N_CORES = 8
BL = B_REAL // N_CORES  # 4 local batch
NT = (N_REAL + 127) // 128        # 79 k tiles
MTS = (NT + N_CORES - 1) // N_CORES * N_CORES  # 80 m tiles (padded to shard)
MS = MTS // N_CORES     # 10 m tiles per core shard
GRP = 4                 # node tiles per linear-stage group

A_SCALE = 4096.0  # fp8 pre-scale for A (avoids subnormal flush)


def build_nc(repeats=1, gather_in_loop=True):
    import concourse.bacc as bacc
    import concourse.mybir as mybir
    import concourse.tile as tile

    f32 = mybir.dt.float32
    bf = mybir.dt.bfloat16
    f8 = mybir.dt.float8e4
    ALU = mybir.AluOpType
    ACT = mybir.ActivationFunctionType

    NTP = NT * 128
    F = BL * G                # 264
    FP = (F + 15) // 16 * 16  # 272 fp8 tile stride (DoubleRow step%16==0)
    inv_s = 1.0 / A_SCALE

    nc = bacc.Bacc(
        "TRN2", target_bir_lowering=False, debug=False, num_devices=N_CORES
    )

    a_t = nc.dram_tensor("a_t", [MS, 128, NT, 128], f8, kind="ExternalInput")
    x0_d = nc.dram_tensor("x0", [NTP, F], bf, kind="ExternalInput")
    x0t_d = nc.dram_tensor("x0t", [BL, G, NTP], bf, kind="ExternalInput")
    wru_d = nc.dram_tensor("wru", [3, G, RU], bf, kind="ExternalInput")
    wc12_d = nc.dram_tensor("wc12", [2, G, H], bf, kind="ExternalInput")
    wcin_d = nc.dram_tensor("wcin", [D_IN, H], bf, kind="ExternalInput")
    wcst_d = nc.dram_tensor("wcst", [H, H], bf, kind="ExternalInput")
    bru_d = nc.dram_tensor("bru", [RU, 1], f32, kind="ExternalInput")
    bc_d = nc.dram_tensor("bc", [H, 1], f32, kind="ExternalInput")
    id_d = nc.dram_tensor("ident", [128, 128], bf, kind="ExternalInput")
    out_d = nc.dram_tensor("outt", [BL, H, NTP], bf, kind="ExternalOutput")
    u_d = nc.dram_tensor("u_i", [BL, H, NTP], bf)
    rs_d = nc.dram_tensor("rs_i", [BL, H, NTP], bf)
    a_bnc = nc.dram_tensor("a_bnc", [MS, 128, NT, 128], f8)
    # A arrives in 2 gathered stripes (halves of each core's shard), so
    # pass 1 can consume stripe 0 while stripe 1 is still in flight.
    # a_gs[s][c*SW + j] = m-tile c*MS + s*SW + j.
    SW = MS // 2
    a_gs = [
        nc.dram_tensor(f"a_g{s}", [N_CORES * SW, 128, NT, 128], f8,
                       addr_space="Shared")
        for s in range(2)
    ]

    def a_tile(m):
        i, c = m % MS, m // MS
        return a_gs[i // SW][c * SW + i % SW]

    groups = [(g, min(GRP, NT - g)) for g in range(0, NT, GRP)]
    # pass-1 visits stripe-0 m-tiles first so compute overlaps stripe 1
    stripe_order = [c * MS + s * SW + j for s in range(2) for j in range(SW)
                    for c in range(N_CORES) if c * MS + s * SW + j < NT]

    with tile.TileContext(nc) as tc:
        with (
            tc.tile_pool(name="persist", bufs=1) as persist,
            tc.tile_pool(name="apool", bufs=3) as apool,
            tc.tile_pool(name="xtpool", bufs=2) as xtpool,
            tc.tile_pool(name="spool", bufs=3) as spool,
            tc.tile_pool(name="pmm", bufs=4, space="PSUM") as pmm,
            tc.tile_pool(name="ptr", bufs=2, space="PSUM") as ptr,
            tc.tile_pool(name="pz", bufs=2, space="PSUM") as pz,
        ):
            x0_buf = persist.tile([128, NT * F], bf, tag="x0b")
            x1_buf = persist.tile([128, NT * F], bf, tag="x1b")
            x0f8_buf = persist.tile([128, NT * FP], f8, tag="x0f8b")
            x1f8_buf = persist.tile([128, NT * FP], f8, tag="x1f8b")
            wru_sb = persist.tile([G, 3 * RU], bf, tag="wru")
            wc12_sb = persist.tile([G, 2 * H], bf, tag="wc12")
            wcin_sb = persist.tile([G, H], bf, tag="wcin")  # rows H:G used
            wcst_sb = persist.tile([H, H], bf, tag="wcst")
            ident = persist.tile([128, 128], bf, tag="ident")
            bru_sb = persist.tile([RU, 1], f32, tag="bru")
            bc_sb = persist.tile([H, 1], f32, tag="bc")

            def load_x0():
                for k in range(NT):
                    nc.sync.dma_start(
                        x0_buf[:, k * F:(k + 1) * F],
                        x0_d[k * 128:(k + 1) * 128, :],
                    )
                    nc.vector.tensor_copy(
                        x0f8_buf[:, k * FP:k * FP + F],
                        x0_buf[:, k * F:(k + 1) * F],
                    )

            def gather_a():
                for s in range(2):
                    nc.sync.dma_start(
                        a_bnc[s * SW:(s + 1) * SW], a_t[s * SW:(s + 1) * SW]
                    )
                    nc.gpsimd.collective_compute(
                        "AllGather", mybir.AluOpType.bypass,
                        replica_groups=[list(range(N_CORES))],
                        ins=[a_bnc[s * SW:(s + 1) * SW].opt()],
                        outs=[a_gs[s][:].opt()],
                    )

            load_x0()
            for m in range(3):
                nc.sync.dma_start(wru_sb[:, m * RU:(m + 1) * RU], wru_d[m])
            for m in range(2):
                nc.sync.dma_start(wc12_sb[:, m * H:(m + 1) * H], wc12_d[m])
            nc.sync.dma_start(wcin_sb[H:G, :], wcin_d[:])
            nc.sync.dma_start(wcst_sb[:], wcst_d[:])
            nc.sync.dma_start(ident[:], id_d[:])
            nc.sync.dma_start(bru_sb[:], bru_d[:])
            nc.sync.dma_start(bc_sb[:], bc_d[:])

            def diffusion_pass(rhs_buf, handler, m_order=None):
                # rhs_buf is the fp8 (FP-strided) shadow buffer
                for m in (m_order if m_order is not None else range(NT)):
                    ps = pmm.tile([128, F], f32, tag="mmps")
                    # whole-m A row: one 1.29MB DMA (78%+ DMA efficiency)
                    at = apool.tile([128, NT * 128], f8, tag="astage")
                    nc.sync.dma_start(at[:], a_tile(m))
                    for kk in range(0, NT - 1, 2):
                        lhsT = at[
                            :, kk * 128:(kk + 2) * 128
                        ].rearrange("p (two m2) -> p two m2", two=2)
                        rhs = rhs_buf[
                            :, kk * FP:(kk + 2) * FP
                        ].rearrange("p (two f) -> p two f", two=2)[
                            :, :, 0:F
                        ]
                        nc.tensor.matmul(
                            ps[:], lhsT, rhs,
                            start=(kk == 0), stop=(kk + 2 == NT),
                            perf_mode=mybir.MatmulPerfMode.DoubleRow,
                        )
                    if NT % 2:
                        kk = NT - 1
                        nc.tensor.matmul(
                            ps[:],
                            at[:, kk * 128:(kk + 1) * 128],
                            rhs_buf[:, kk * FP:kk * FP + F],
                            start=(kk == 0),
                            stop=True,
                        )
                    handler(m, ps)

            def h_copy(m, ps):
                nc.vector.tensor_scalar_mul(
                    x1_buf[:, m * F:(m + 1) * F], ps[:], inv_s
                )
                nc.vector.tensor_copy(
                    x1f8_buf[:, m * FP:m * FP + F],
                    x1_buf[:, m * F:(m + 1) * F],
                )

            stage = {}

            def transpose_lo_hi(m, j, lo_buf_slice, hi_tile):
                # lo = previous diffusion output [128, F]; hi = 2*A@lo - base
                for b in range(BL):
                    tp = ptr.tile([128, 128], bf, tag="trp")
                    nc.tensor.transpose(
                        tp[:G, :], lo_buf_slice[:, b * G:(b + 1) * G], ident[:]
                    )
                    nc.vector.tensor_copy(
                        stage[(1, b)][:, j * 128:(j + 1) * 128], tp[:G, :]
                    )
                    tp2 = ptr.tile([128, 128], bf, tag="trp")
                    nc.tensor.transpose(
                        tp2[:G, :], hi_tile[:, b * G:(b + 1) * G], ident[:]
                    )
                    nc.vector.tensor_copy(
                        stage[(2, b)][:, j * 128:(j + 1) * 128], tp2[:G, :]
                    )

            def h_gconv1(m, ps):
                gi, j = m // GRP, m % GRP
                g0, gn = groups[gi]
                if j == 0:
                    for b in range(BL):
                        stage[(1, b)] = xtpool.tile(
                            [G, GRP * 128], bf, tag=f"s1_{b}", name=f"s1_{b}"
                        )
                        stage[(2, b)] = xtpool.tile(
                            [G, GRP * 128], bf, tag=f"s2_{b}", name=f"s2_{b}"
                        )
                x2t = spool.tile([128, F], bf, tag="x2tmp")
                nc.vector.scalar_tensor_tensor(
                    x2t[:], ps[:], 2.0 * inv_s, x0_buf[:, m * F:(m + 1) * F],
                    op0=ALU.mult, op1=ALU.subtract,
                )
                transpose_lo_hi(m, j, x1_buf[:, m * F:(m + 1) * F], x2t)
                if j == gn - 1:
                    W = gn * 128
                    c0 = g0 * 128
                    for b in range(BL):
                        x0t_t = spool.tile([G, GRP * 128], bf, tag="x0ts")
                        nc.sync.dma_start(x0t_t[:, :W], x0t_d[b, :, c0:c0 + W])
                        zp = pz.tile([128, 512], f32, tag="zps")
                        nc.tensor.matmul(
                            zp[:, :W], wru_sb[:, 0:RU], x0t_t[:, :W],
                            start=True, stop=False,
                        )
                        nc.tensor.matmul(
                            zp[:, :W], wru_sb[:, RU:2 * RU],
                            stage[(1, b)][:, :W], start=False, stop=False,
                        )
                        nc.tensor.matmul(
                            zp[:, :W], wru_sb[:, 2 * RU:3 * RU],
                            stage[(2, b)][:, :W], start=False, stop=True,
                        )
                        rt = spool.tile([H, GRP * 128], bf, tag="rt")
                        nc.scalar.activation(
                            rt[:, :W], zp[0:H, :W], ACT.Sigmoid,
                            bias=bru_sb[0:H],
                        )
                        ut = spool.tile([H, GRP * 128], bf, tag="ut")
                        nc.scalar.activation(
                            ut[:, :W], zp[H:RU, :W], ACT.Sigmoid,
                            bias=bru_sb[H:RU],
                        )
                        nc.sync.dma_start(u_d[b, :, c0:c0 + W], ut[:, :W])
                        rs = spool.tile([H, GRP * 128], bf, tag="rs")
                        nc.vector.tensor_mul(
                            rs[:, :W], rt[:, :W], x0t_t[0:H, :W]
                        )
                        nc.sync.dma_start(rs_d[b, :, c0:c0 + W], rs[:, :W])
                        for jj in range(gn):
                            m2 = g0 + jj
                            tpb = ptr.tile([128, 128], bf, tag="trp")
                            nc.tensor.transpose(
                                tpb[:, :H], rs[:, jj * 128:(jj + 1) * 128],
                                ident[:H, :H],
                            )
                            nc.vector.tensor_copy(
                                x0_buf[:, m2 * F + b * G:
                                       m2 * F + b * G + H],
                                tpb[:, :H],
                            )
                            nc.vector.tensor_copy(
                                x0f8_buf[:, m2 * FP + b * G:
                                         m2 * FP + b * G + H],
                                tpb[:, :H],
                            )

            def h_gconv2(m, ps):
                gi, j = m // GRP, m % GRP
                g0, gn = groups[gi]
                if j == 0:
                    for b in range(BL):
                        stage[(1, b)] = xtpool.tile(
                            [G, GRP * 128], bf, tag=f"s1_{b}", name=f"s1_{b}"
                        )
                        stage[(2, b)] = xtpool.tile(
                            [G, GRP * 128], bf, tag=f"s2_{b}", name=f"s2_{b}"
                        )
                y2t = spool.tile([128, F], bf, tag="x2tmp")
                nc.vector.scalar_tensor_tensor(
                    y2t[:], ps[:], 2.0 * inv_s, x0_buf[:, m * F:(m + 1) * F],
                    op0=ALU.mult, op1=ALU.subtract,
                )
                transpose_lo_hi(m, j, x1_buf[:, m * F:(m + 1) * F], y2t)
                if j == gn - 1:
                    W = gn * 128
                    c0 = g0 * 128
                    for b in range(BL):
                        x0t_t = spool.tile([G, GRP * 128], bf, tag="x0ts")
                        nc.sync.dma_start(x0t_t[:, :W], x0t_d[b, :, c0:c0 + W])
                        rs_t = spool.tile([H, GRP * 128], bf, tag="rsin")
                        nc.sync.dma_start(rs_t[:, :W], rs_d[b, :, c0:c0 + W])
                        zc = pz.tile([128, 512], f32, tag="zps")
                        nc.tensor.matmul(
                            zc[:H, :W], wcin_sb[H:G, :], x0t_t[H:G, :W],
                            start=True, stop=False,
                        )
                        nc.tensor.matmul(
                            zc[:H, :W], wcst_sb[:], rs_t[:, :W],
                            start=False, stop=False,
                        )
                        nc.tensor.matmul(
                            zc[:H, :W], wc12_sb[:, 0:H],
                            stage[(1, b)][:, :W], start=False, stop=False,
                        )
                        nc.tensor.matmul(
                            zc[:H, :W], wc12_sb[:, H:2 * H],
                            stage[(2, b)][:, :W], start=False, stop=True,
                        )
                        ct = spool.tile([H, GRP * 128], f32, tag="ct")
                        nc.scalar.activation(
                            ct[:, :W], zc[:H, :W], ACT.Tanh, bias=bc_sb[:]
                        )
                        ut2 = spool.tile([H, GRP * 128], bf, tag="uin")
                        nc.sync.dma_start(ut2[:, :W], u_d[b, :, c0:c0 + W])
                        t1 = spool.tile([H, GRP * 128], f32, tag="t1")
                        nc.vector.tensor_sub(
                            t1[:, :W], x0t_t[0:H, :W], ct[:, :W]
                        )
                        nc.vector.tensor_mul(t1[:, :W], t1[:, :W], ut2[:, :W])
                        op = spool.tile([H, GRP * 128], bf, tag="outp")
                        nc.vector.tensor_add(op[:, :W], t1[:, :W], ct[:, :W])
                        nc.sync.dma_start(out_d[b, :, c0:c0 + W], op[:, :W])

            if not gather_in_loop:
                gather_a()
            for rep in range(repeats):
                if gather_in_loop:
                    gather_a()
                if rep > 0:
                    load_x0()                      # x0_buf was turned into y0
                diffusion_pass(x0f8_buf, h_copy, m_order=stripe_order)
                                                   # pass 1: x1 = A x0
                diffusion_pass(x1f8_buf, h_gconv1) # pass 2: x2 + gconv1 linear
                diffusion_pass(x0f8_buf, h_copy)   # pass 3: y1 = A y0
                diffusion_pass(x1f8_buf, h_gconv2) # pass 4: y2 + gconv2 + out

    nc.compile()
    return nc


def _f8_lut():
    """LUT mapping bf16 bit patterns -> fp8 bits of (value * A_SCALE)."""
    v = np.arange(65536, dtype=np.uint16).view(BF16).astype(np.float32)
    with np.errstate(invalid="ignore", over="ignore"):
        return (v * A_SCALE).astype(F8).view(np.uint8)


def prep_a(supports):
    """Full A -> fp8, transposed + tiled: a[m,p,k,q] = A[m*128+q, k*128+p]."""
    N = supports.shape[0]
    lut = _f8_lut()
    a_bf = supports.astype(BF16)                      # fast truncation cast
    a8 = lut[a_bf.view(np.uint16)]                    # fp8 bits, row-major A
    ap = np.zeros((MTS * 128, NT * 128), np.uint8)    # pad rows(m) and cols(k)
    ap[:N, :N] = a8
    out = np.empty((MTS, 128, NT, 128), np.uint8)
    for m in range(MTS):                              # blocked: ~8x faster
        out[m] = ap[m * 128:(m + 1) * 128].reshape(128, NT, 128).transpose(2, 1, 0)
    return out.view(F8)


def prep_shared(W_ru, b_ru, W_c, b_c):
    # feature order is [states(64); inputs(2)] throughout the kernel
    perm = list(range(D_IN, G)) + list(range(D_IN))
    wru_r = np.ascontiguousarray(
        W_ru.reshape(G, 3, RU)[perm].transpose(1, 0, 2)
    ).astype(BF16)
    wc_r = np.ascontiguousarray(
        W_c.reshape(G, 3, H)[perm].transpose(1, 0, 2)
    ).astype(BF16)
    return {
        "wru": wru_r,
        "wc12": np.ascontiguousarray(wc_r[1:3]),
        "wcin": np.ascontiguousarray(wc_r[0, H:G]),
        "wcst": np.ascontiguousarray(wc_r[0, 0:H]),
        "bru": b_ru.reshape(RU, 1).astype(np.float32),
        "bc": b_c.reshape(H, 1).astype(np.float32),
        "ident": np.eye(128, dtype=BF16),
    }


def prep_core(x_cat):
    # x_cat: [BL, N, G] f32 for this core's batch slice
    NTP = NT * 128
    N = x_cat.shape[1]
    F = BL * G
    x0 = np.zeros((NTP, F), BF16)
    x0[:N] = x_cat.transpose(1, 0, 2).reshape(N, F)
    x0t = np.zeros((BL, G, NTP), BF16)
    x0t[:, :, :N] = x_cat.transpose(0, 2, 1)
    return {"x0": x0, "x0t": x0t}


_NC_CACHE = {}


def _get_nc(repeats=1):
    if repeats not in _NC_CACHE:
        _NC_CACHE[repeats] = build_nc(repeats=repeats)
    return _NC_CACHE[repeats]


def make_in_maps(inputs, states, supports, W_ru, b_ru, W_c, b_c):
    inputs = np.asarray(inputs, np.float32)
    states = np.asarray(states, np.float32)
    supports = np.asarray(supports, np.float32)
    a_full = prep_a(supports)
    shared = prep_shared(
        np.asarray(W_ru, np.float32), np.asarray(b_ru, np.float32),
        np.asarray(W_c, np.float32), np.asarray(b_c, np.float32),
    )
    x_cat = np.concatenate([states, inputs], -1)  # [B, N, G], states-first
    in_maps = []
    for c in range(N_CORES):
        m = dict(shared)
        m["a_t"] = a_full[c * MS:(c + 1) * MS]
        m.update(prep_core(x_cat[c * BL:(c + 1) * BL]))
        in_maps.append(m)
    return in_maps


def _run(inputs, states, supports, W_ru, b_ru, W_c, b_c):
    from concourse.bass_utils import run_bass_kernel_spmd

    B, N, _ = np.asarray(inputs).shape
    nc = _get_nc(repeats=1)
    in_maps = make_in_maps(inputs, states, supports, W_ru, b_ru, W_c, b_c)
    res = run_bass_kernel_spmd(nc, in_maps, list(range(N_CORES)))
    out = np.empty((B, N, H), np.float32)
    for c in range(N_CORES):
        o = res.results[c]["outt"]          # [BL, H, NTP] bf16
        out[c * BL:(c + 1) * BL] = o.transpose(0, 2, 1)[:, :N, :].astype(np.float32)
    return out, res


def kernel(**kw):
    out, _ = _run(
        kw["inputs"], kw["states"], kw["supports"],
        kw["W_ru"], kw["b_ru"], kw["W_c"], kw["b_c"],
    )
    return out
